# revision 3
# baseline (speedup 1.0000x reference)
"""Trainium2 Bass kernel: 3x3 SAME conv (stride 1), NCHW fp32.

Problem: image [32, 64, 112, 112] * weight [64, 64, 3, 3] + bias [64]
Sharding: data-parallel over batch across 8 NeuronCores (4 images each).

Per-core strategy (v2 — "parity dual-form"):
  - SBUF image layout is parity-split: partition p = 64*s + cin, where
    s=0 holds the EVEN padded rows (rows 0,2,..,112 of the 114x114
    zero-padded image) and s=1 the ODD rows (1,3,..,113), both packed
    consecutively at the same per-partition offsets. At any per-partition
    offset x, partitions 64..127 hold the row exactly one below the row
    at partitions 0..63 — a built-in +1-row shift with zero duplication.
  - Each matmul then uses the FULL array: K=128 (two input rows x 64 cin)
    and M=128 (two output rows x 64 cout). For an output row pair
    (r1, r2) = (2t, 2t+1), six matmuls cover all 9 taps of both rows:
      MM_A(kw): rhs rows (2t, 2t+1),   lhsT quadrants [w0 | 0 ; w1 | w0]
      MM_B(kw): rhs rows (2t+2, 2t+3), lhsT quadrants [w2 | w1; 0  | w2]
    (w_kh = w[:, :, kh, kw] as [cin, cout]); kw in {0,1,2} is a column
    offset into the padded rows. 18 tap-contributions / 6 matmuls.
  - One matmul streams 4 row-pairs (8 output rows): rhs AP
    [[WP, 4], [1, 112]] (pair stride = one parity-packed row), N=448
    columns into a single PSUM bank; 6 accumulating matmuls per tile.
    Per image: 14 tiles x 6 MMs; per core 336 matmuls of N=448.
  - Drain: one tensor op per PSUM tile (alternating DVE / Activation
    engine to split the load), fused +bias, partition-aligned into a
    stage tile. Output HBM tensor is parity-split [n, 2, 64, 56, 112]
    (host reassembles NCHW), so every stage partition is one contiguous
    HBM run — two PSUM tiles per 3584B-run output DMA.
  - Host-side prep packs the image parity-split ([n, 2, 64, 57, 114]
    bf16) so the input DMA is one 128-partition contiguous-run transfer
    per image.
"""

import numpy as np

import concourse.bass as bass
import concourse.mybir as mybir
import concourse.tile as tile
from concourse import bacc, bass_utils

N_CORES = 8
IMGS = 4  # images per core
CIN = 64
COUT = 64
H = 112
W = 112
HP = H + 2  # 114
WP = W + 2  # 114
EROWS = 57  # parity-packed rows per half (57 even / 57 odd)
F = EROWS * WP  # 6498 elems per partition
PAIRS = 56  # output row pairs per image
TP = 4  # row pairs per PSUM tile
NT = PAIRS // TP  # 14 PSUM tiles per image
NMM = TP * W  # 448 matmul free size

F32 = mybir.dt.float32
BF16 = mybir.dt.bfloat16


def _ap(ap_obj, offset, dims):
    """Manual AP on the same tensor handle; dims = [[step, count], ...]."""
    return bass.AP(tensor=ap_obj.tensor, offset=offset, ap=dims)


def build_nc(n_imgs=IMGS):
    nc = bacc.Bacc(
        "TRN2",
        target_bir_lowering=False,
        debug=False,
        num_devices=N_CORES,
    )
    img_d = nc.dram_tensor("image_par", [n_imgs, 2, CIN, EROWS, WP], BF16, kind="ExternalInput")
    wt_d = nc.dram_tensor("weight2", [128, 6 * 128], BF16, kind="ExternalInput")
    bias_d = nc.dram_tensor("bias2", [128, 1], F32, kind="ExternalInput")
    out_d = nc.dram_tensor("out_par", [n_imgs, 2, COUT, PAIRS, W], F32, kind="ExternalOutput")

    img_ap = img_d.ap()
    out_ap = out_d.ap()

    with tile.TileContext(nc) as tc:
        with (
            tc.tile_pool(name="img", bufs=2) as img_pool,
            tc.tile_pool(name="wt", bufs=1) as wt_pool,
            tc.tile_pool(name="bias", bufs=1) as bias_pool,
            tc.tile_pool(name="stage", bufs=4) as stage_pool,
            tc.tile_pool(name="psum", bufs=4, space="PSUM") as psum_pool,
        ):
            wt_t = wt_pool.tile([128, 6 * 128], BF16)
            nc.sync.dma_start(wt_t[:], wt_d.ap()[:])
            bias_t = bias_pool.tile([128, 1], F32)
            nc.sync.dma_start(bias_t[:], bias_d.ap()[:])

            drain_flip = 0
            for n in range(n_imgs):
                img_t = img_pool.tile([128, F], BF16)
                # one 128-partition DMA: partition 64*s + cin <- parity-
                # packed rows of channel cin, 57*114 bf16 contiguous.
                src = _ap(
                    img_ap,
                    n * 2 * CIN * F,
                    [[CIN * F, 2], [F, CIN], [1, F]],
                )
                nc.sync.dma_start(img_t[:], src)

                for q in range(NT // 2):
                    stg = stage_pool.tile([128, 2 * NMM], F32)
                    for h in range(2):
                        t0 = (2 * q + h) * TP  # first pair of this tile
                        ps = psum_pool.tile([128, NMM], F32)
                        img_full = img_t[:]
                        for j in range(6):
                            kw, mtype = divmod(j, 2)
                            base = (t0 + mtype) * WP + kw
                            rhs = bass.AP(
                                tensor=img_full.tensor,
                                offset=img_full.offset + base,
                                ap=[[F, 128], [WP, TP], [1, W]],
                            )
                            lhsT = wt_t[:, j * 128 : (j + 1) * 128]
                            nc.tensor.matmul(
                                ps[:], lhsT, rhs,
                                start=(j == 0), stop=(j == 5),
                            )
                        # drain + bias, alternating engine
                        dst = stg[:, h * NMM : (h + 1) * NMM]
                        if drain_flip & 1:
                            nc.vector.tensor_scalar_add(dst, ps[:], bias_t[:])
                        else:
                            nc.scalar.add(dst, ps[:], bias_t[:])
                        drain_flip += 1
                    # 2 PSUM tiles -> one DMA; per partition (s, cout) the
                    # 8 pairs t0..t0+7 are 2*448 contiguous fp32 in out_par.
                    g0 = 2 * q * TP  # first pair of the group
                    dst_d = _ap(
                        out_ap,
                        n * 2 * COUT * PAIRS * W + g0 * W,
                        [[COUT * PAIRS * W, 2], [PAIRS * W, COUT], [1, 2 * NMM]],
                    )
                    nc.sync.dma_start(dst_d, stg[:])

    nc.compile()
    return nc


_NC_CACHE = {}


def _get_nc(n_imgs=IMGS):
    if n_imgs not in _NC_CACHE:
        _NC_CACHE[n_imgs] = build_nc(n_imgs)
    return _NC_CACHE[n_imgs]


def _prep_inputs(image, weight, bias):
    import ml_dtypes

    image = np.asarray(image, dtype=np.float32)
    weight = np.asarray(weight, dtype=np.float32)
    bias = np.asarray(bias, dtype=np.float32)
    n = image.shape[0]
    bf16 = ml_dtypes.bfloat16
    pad = np.zeros((n, CIN, HP, WP), bf16)
    pad[:, :, 1 : 1 + H, 1 : 1 + W] = image.astype(bf16)
    # parity split: [n, s, cin, e, col], s=0 even rows, s=1 odd rows
    img_par = np.ascontiguousarray(
        pad.reshape(n, CIN, EROWS, 2, WP).transpose(0, 3, 1, 2, 4)
    )
    # lhsT blocks j = 2*kw + {0:A, 1:B}; quadrant [k, m], k=64*sK+cin,
    # m=64*sM+cout. wT[kh] = w[:, :, kh, kw] as [cin, cout].
    wT = weight.transpose(1, 0, 2, 3)  # [cin, cout, kh, kw]
    blocks = np.zeros((6, 128, 128), np.float32)
    for kw in range(3):
        a, b = blocks[2 * kw], blocks[2 * kw + 1]
        a[0:64, 0:64] = wT[:, :, 0, kw]
        a[64:128, 0:64] = wT[:, :, 1, kw]
        a[64:128, 64:128] = wT[:, :, 0, kw]
        b[0:64, 0:64] = wT[:, :, 2, kw]
        b[0:64, 64:128] = wT[:, :, 1, kw]
        b[64:128, 64:128] = wT[:, :, 2, kw]
    wt2 = np.ascontiguousarray(blocks.transpose(1, 0, 2).reshape(128, 6 * 128)).astype(bf16)
    b2 = np.concatenate([bias, bias]).reshape(128, 1)
    return img_par, wt2, b2


def run_cores(image, weight, bias, trace=False, **kw):
    """Shard over 8 cores, run, return (full_output, BassKernelResults)."""
    img_par, wt2, b2 = _prep_inputs(image, weight, bias)
    n = img_par.shape[0]
    per = n // N_CORES
    assert per * N_CORES == n
    nc = _get_nc(per)
    in_maps = [
        {
            "image_par": np.ascontiguousarray(img_par[i * per : (i + 1) * per]),
            "weight2": wt2,
            "bias2": b2,
        }
        for i in range(N_CORES)
    ]
    res = bass_utils.run_bass_kernel_spmd(
        nc, in_maps, core_ids=list(range(N_CORES)), trace=trace, **kw
    )
    # out_par [per, 2, COUT, 56, W] -> [per, COUT, 112, W]
    outs = []
    for i in range(N_CORES):
        op = res.results[i]["out_par"]
        outs.append(op.transpose(0, 2, 3, 1, 4).reshape(per, COUT, H, W))
    out = np.concatenate(outs, axis=0)
    return out, res


def kernel(image, weight, bias):
    out, _ = run_cores(image, weight, bias, trace=False)
    return out


# revision 6
# speedup vs baseline: 1.4164x; 1.4164x over previous
"""Trainium2 Bass kernel: 3x3 SAME conv (stride 1), NCHW fp32.

Problem: image [32, 64, 112, 112] * weight [64, 64, 3, 3] + bias [64]
Sharding: data-parallel over batch across 8 NeuronCores (4 images each).

Per-core strategy (v2 — "parity dual-form"):
  - SBUF image layout is parity-split: partition p = 64*s + cin, where
    s=0 holds the EVEN padded rows (rows 0,2,..,112 of the 114x114
    zero-padded image) and s=1 the ODD rows (1,3,..,113), both packed
    consecutively at the same per-partition offsets. At any per-partition
    offset x, partitions 64..127 hold the row exactly one below the row
    at partitions 0..63 — a built-in +1-row shift with zero duplication.
  - Each matmul then uses the FULL array: K=128 (two input rows x 64 cin)
    and M=128 (two output rows x 64 cout). For an output row pair
    (r1, r2) = (2t, 2t+1), six matmuls cover all 9 taps of both rows:
      MM_A(kw): rhs rows (2t, 2t+1),   lhsT quadrants [w0 | 0 ; w1 | w0]
      MM_B(kw): rhs rows (2t+2, 2t+3), lhsT quadrants [w2 | w1; 0  | w2]
    (w_kh = w[:, :, kh, kw] as [cin, cout]); kw in {0,1,2} is a column
    offset into the padded rows. 18 tap-contributions / 6 matmuls.
  - One matmul streams 4 row-pairs (8 output rows): rhs AP
    [[WP, 4], [1, 112]] (pair stride = one parity-packed row), N=448
    columns into a single PSUM bank; 6 accumulating matmuls per tile.
    Per image: 14 tiles x 6 MMs; per core 336 matmuls of N=448.
  - Drain: one tensor op per PSUM tile (alternating DVE / Activation
    engine to split the load), fused +bias, partition-aligned into a
    stage tile. Output HBM tensor is parity-split [n, 2, 64, 56, 112]
    (host reassembles NCHW), so every stage partition is one contiguous
    HBM run — two PSUM tiles per 3584B-run output DMA.
  - Host-side prep packs the image parity-split ([n, 2, 64, 57, 114]
    bf16) so the input DMA is one 128-partition contiguous-run transfer
    per image.
"""

import numpy as np

import concourse.bass as bass
import concourse.mybir as mybir
import concourse.tile as tile
from concourse import bacc, bass_utils

N_CORES = 8
IMGS = 4  # images per core
CIN = 64
COUT = 64
H = 112
W = 112
HP = H + 2  # 114
WP = W + 2  # 114
EROWS = 57  # parity-packed rows per half (57 even / 57 odd)
F = EROWS * WP  # 6498 elems per partition
PAIRS = 56  # output row pairs per image
TP = 4  # row pairs per PSUM tile
NT = PAIRS // TP  # 14 PSUM tiles per image
NMM = TP * W  # 448 matmul free size

F32 = mybir.dt.float32
BF16 = mybir.dt.bfloat16


def _ap(ap_obj, offset, dims):
    """Manual AP on the same tensor handle; dims = [[step, count], ...]."""
    return bass.AP(tensor=ap_obj.tensor, offset=offset, ap=dims)


def build_nc(n_imgs=IMGS):
    nc = bacc.Bacc(
        "TRN2",
        target_bir_lowering=False,
        debug=False,
        num_devices=N_CORES,
    )
    img_d = nc.dram_tensor("image_par", [n_imgs, 2, CIN, EROWS, WP], BF16, kind="ExternalInput")
    wt_d = nc.dram_tensor("weight2", [128, 6 * 128], BF16, kind="ExternalInput")
    bias_d = nc.dram_tensor("bias2", [128, 1], F32, kind="ExternalInput")
    out_d = nc.dram_tensor("out_par", [n_imgs, 2, COUT, PAIRS, W], F32, kind="ExternalOutput")

    img_ap = img_d.ap()
    out_ap = out_d.ap()

    with tile.TileContext(nc) as tc:
        with (
            tc.tile_pool(name="img", bufs=3) as img_pool,
            tc.tile_pool(name="wt", bufs=1) as wt_pool,
            tc.tile_pool(name="bias", bufs=1) as bias_pool,
            tc.tile_pool(name="stage", bufs=6) as stage_pool,
            tc.tile_pool(name="psum", bufs=8, space="PSUM") as psum_pool,
        ):
            wt_t = wt_pool.tile([128, 6 * 128], BF16)
            nc.sync.dma_start(wt_t[:], wt_d.ap()[:])
            bias_t = bias_pool.tile([128, 1], F32)
            nc.sync.dma_start(bias_t[:], bias_d.ap()[:])

            drain_flip = 0
            for n in range(n_imgs):
                img_t = img_pool.tile([128, F], BF16)
                # 128-partition DMAs: partition 64*s + cin <- parity-
                # packed rows of channel cin, contiguous per partition.
                # Chunked along rows so the first matmuls gate on a small
                # transfer (range-based deps let tiles start early).
                for r0, r1 in ((0, 8), (8, 32), (32, EROWS)):
                    src = _ap(
                        img_ap,
                        n * 2 * CIN * F + r0 * WP,
                        [[CIN * F, 2], [F, CIN], [1, (r1 - r0) * WP]],
                    )
                    nc.sync.dma_start(img_t[:, r0 * WP : r1 * WP], src)

                for q in range(NT // 2):
                    stg = stage_pool.tile([128, 2 * NMM], F32)
                    for h in range(2):
                        t0 = (2 * q + h) * TP  # first pair of this tile
                        ps = psum_pool.tile([128, NMM], F32)
                        img_full = img_t[:]
                        for j in range(6):
                            kw, mtype = divmod(j, 2)
                            base = (t0 + mtype) * WP + kw
                            rhs = bass.AP(
                                tensor=img_full.tensor,
                                offset=img_full.offset + base,
                                ap=[[F, 128], [WP, TP], [1, W]],
                            )
                            lhsT = wt_t[:, j * 128 : (j + 1) * 128]
                            nc.tensor.matmul(
                                ps[:], lhsT, rhs,
                                start=(j == 0), stop=(j == 5),
                            )
                        # drain + bias, alternating engine
                        dst = stg[:, h * NMM : (h + 1) * NMM]
                        if drain_flip & 1:
                            nc.vector.tensor_scalar_add(dst, ps[:], bias_t[:])
                        else:
                            nc.scalar.add(dst, ps[:], bias_t[:])
                        drain_flip += 1
                    # 2 PSUM tiles -> one DMA; per partition (s, cout) the
                    # 8 pairs t0..t0+7 are 2*448 contiguous fp32 in out_par.
                    g0 = 2 * q * TP  # first pair of the group
                    dst_d = _ap(
                        out_ap,
                        n * 2 * COUT * PAIRS * W + g0 * W,
                        [[COUT * PAIRS * W, 2], [PAIRS * W, COUT], [1, 2 * NMM]],
                    )
                    # gpsimd (SWDGE) queue: keeps output DMAs off the SP
                    # sequencer so input prefetch is never head-of-line
                    # blocked behind drain-gated output transfers.
                    nc.gpsimd.dma_start(dst_d, stg[:])

    nc.compile()
    return nc


_NC_CACHE = {}


def _get_nc(n_imgs=IMGS):
    if n_imgs not in _NC_CACHE:
        _NC_CACHE[n_imgs] = build_nc(n_imgs)
    return _NC_CACHE[n_imgs]


def _prep_inputs(image, weight, bias):
    import ml_dtypes

    image = np.asarray(image, dtype=np.float32)
    weight = np.asarray(weight, dtype=np.float32)
    bias = np.asarray(bias, dtype=np.float32)
    n = image.shape[0]
    bf16 = ml_dtypes.bfloat16
    pad = np.zeros((n, CIN, HP, WP), bf16)
    pad[:, :, 1 : 1 + H, 1 : 1 + W] = image.astype(bf16)
    # parity split: [n, s, cin, e, col], s=0 even rows, s=1 odd rows
    img_par = np.ascontiguousarray(
        pad.reshape(n, CIN, EROWS, 2, WP).transpose(0, 3, 1, 2, 4)
    )
    # lhsT blocks j = 2*kw + {0:A, 1:B}; quadrant [k, m], k=64*sK+cin,
    # m=64*sM+cout. wT[kh] = w[:, :, kh, kw] as [cin, cout].
    wT = weight.transpose(1, 0, 2, 3)  # [cin, cout, kh, kw]
    blocks = np.zeros((6, 128, 128), np.float32)
    for kw in range(3):
        a, b = blocks[2 * kw], blocks[2 * kw + 1]
        a[0:64, 0:64] = wT[:, :, 0, kw]
        a[64:128, 0:64] = wT[:, :, 1, kw]
        a[64:128, 64:128] = wT[:, :, 0, kw]
        b[0:64, 0:64] = wT[:, :, 2, kw]
        b[0:64, 64:128] = wT[:, :, 1, kw]
        b[64:128, 64:128] = wT[:, :, 2, kw]
    wt2 = np.ascontiguousarray(blocks.transpose(1, 0, 2).reshape(128, 6 * 128)).astype(bf16)
    b2 = np.concatenate([bias, bias]).reshape(128, 1)
    return img_par, wt2, b2


def run_cores(image, weight, bias, trace=False, **kw):
    """Shard over 8 cores, run, return (full_output, BassKernelResults)."""
    img_par, wt2, b2 = _prep_inputs(image, weight, bias)
    n = img_par.shape[0]
    per = n // N_CORES
    assert per * N_CORES == n
    nc = _get_nc(per)
    in_maps = [
        {
            "image_par": np.ascontiguousarray(img_par[i * per : (i + 1) * per]),
            "weight2": wt2,
            "bias2": b2,
        }
        for i in range(N_CORES)
    ]
    res = bass_utils.run_bass_kernel_spmd(
        nc, in_maps, core_ids=list(range(N_CORES)), trace=trace, **kw
    )
    # out_par [per, 2, COUT, 56, W] -> [per, COUT, 112, W]
    outs = []
    for i in range(N_CORES):
        op = res.results[i]["out_par"]
        outs.append(op.transpose(0, 2, 3, 1, 4).reshape(per, COUT, H, W))
    out = np.concatenate(outs, axis=0)
    return out, res


def kernel(image, weight, bias):
    out, _ = run_cores(image, weight, bias, trace=False)
    return out
